# revision 11
# baseline (speedup 1.0000x reference)
"""AdaptiveSparsityAttention TRN2 kernel (8 NeuronCores, SPMD data-parallel).

Problem (B=2, S=1024, D=512, H=2 heads, dh=256, hidden=128):
  q,k,v = x@Wq, x@Wk, x@Wv (split 2 heads); scores = q@k^T/16
  a_i = q_mean@W1[:dh]+b1, c_j = k_mean@W1[dh:]
  z[i,j] = W2 . relu(a_i + c_j)          (sigmoid(z+b2)>0.5  <=>  z > -b2)
  attn = softmax(mask(scores));  out = (attn@v)@Wo + bo

Sharding: 8 cores = 2 batches x 4 query-chunks of 256 rows. Each core
computes its output chunk fully locally (K/V/k_mean recomputed per core
from its batch's x - cheap), no collectives.

Key structure per core:
  - a/c computed EXACTLY (fp32) via host-folded Mq = Wq_mean@W1[:dh],
    Mk = Wk_mean@W1[dh:]  (z margins are tiny: fp32 mandatory for mask)
  - T_i = relu(C^T + a_i) as [128h x 1024j] tiles, produced by
    DVE tensor_scalar / ACT activation(Relu,bias) / GPSIMD in rotation
  - z rows assembled in PSUM via delta-trick: accumulate matmuls with
    stationary w*e_i (sliding slice of a host-built buffer); 4-way
    col-tiling (tile_position) lets 4 query rows stream concurrently
  - mask applied as multiply on exp(scores - rowmax); row max over
    UNMASKED scores (mathematically identical softmax)
  - scores/AV/projections in float32r (1 cyc/row), z matmul in fp32
"""

import sys

if "/opt/trn_rl_repo" not in sys.path:
    sys.path.insert(0, "/opt/trn_rl_repo")

import numpy as np

import concourse.bass as bass  # noqa: F401
import concourse.tile as tile
from concourse import bacc, mybir
from concourse.bass_utils import run_bass_kernel_spmd
from concourse.masks import make_identity

F32 = mybir.dt.float32
F32R = mybir.dt.float32r
AL = mybir.AluOpType
AF = mybir.ActivationFunctionType

B, S, D = 2, 1024, 512
DH = D // 2          # 256 per-head dim
HID = 128            # predictor hidden
NCHUNK = S // 4      # 256 query rows per core
P = 128

# knobs (test.py may override before first kernel() call)
CONFIG = {
    "coltile": True,       # 4-way col-tiled z matmuls
    "trace": False,
    "tmpdir": None,
    # T-producer rotation per 16 rows: 'v'=DVE, 's'=ACT, 'g'=GPSIMD
    "tpat": ["v", "s", "v", "s", "v", "g", "v", "s", "v", "s", "v", "g", "v", "s", "v", "g"],
    "t_bufs": 8,
}

_STATE = {}


def _emit(tc, nc, t):
    sl512 = [slice(0, 512), slice(512, 1024)]

    with tc.tile_pool(name="big", bufs=1) as big:
        # ---- persistent residents ----
        mq_s = big.tile([P, 4, HID], F32, name="mq_s")
        nc.sync.dma_start(mq_s[:], t["mq"].rearrange("(t p) h -> p t h", p=P))
        mk_s = big.tile([P, 4, HID], F32, name="mk_s")
        nc.sync.dma_start(mk_s[:], t["mk"].rearrange("(t p) h -> p t h", p=P))
        b1_s = big.tile([P, 1], F32, name="b1_s")
        nc.sync.dma_start(b1_s[:], t["b1c"])
        thr_s = big.tile([P, 1], F32, name="thr_s")
        nc.sync.dma_start(thr_s[:], t["thr"])
        wsel_s = big.tile([P, 64], F32, name="wsel_s")
        nc.sync.dma_start(wsel_s[:], t["wsel32"])
        if not CONFIG["coltile"]:
            wself_s = big.tile([P, 256], F32, name="wself_s")
            nc.sync.dma_start(wself_s[:], t["wself"])
        bo_s = big.tile([1, D], F32, name="bo_s")
        nc.sync.dma_start(bo_s[:], t["bo2"])
        one_s = big.tile([1, P], F32, name="one_s")
        nc.sync.dma_start(one_s[:], t["one1"])

        at_s = big.tile([P, NCHUNK], F32, name="at_s")    # a^T + b1, [h, i]
        ct_s = big.tile([P, S], F32, name="ct_s")          # c^T, [h, j]
        xqr_s = big.tile([P, 4, NCHUNK], F32R, name="xqr_s")
        xtr_s = big.tile([P, 4, S], F32R, name="xtr_s")
        wqr_s = big.tile([P, 4, D], F32R, name="wqr_s")
        wkr_s = big.tile([P, 4, D], F32R, name="wkr_s")
        wvr_s = big.tile([P, 4, D], F32R, name="wvr_s")
        wor_s = big.tile([P, 4, D], F32R, name="wor_s")
        qt_s = big.tile([P, 4, NCHUNK], F32R, name="qt_s")  # q^T/16 [dout, i]
        kt_s = big.tile([P, 4, S], F32R, name="kt_s")       # k^T [dout, j]
        v_s = big.tile([P, 8, D], F32R, name="v_s")         # v [j(8 tiles), d]
        ident = big.tile([P, P], F32, name="ident")
        make_identity(nc, ident[:])
        bor_s = big.tile([1, D], F32R, name="bor_s")
        nc.vector.tensor_copy(bor_s[:], bo_s[:])
        oner_s = big.tile([1, P], F32R, name="oner_s")
        nc.vector.tensor_copy(oner_s[:], one_s[:])
        otr_s = big.tile([P, 4, NCHUNK], F32R, name="otr_s")  # out^T [d, i]

        with (
            tc.tile_pool(name="ps", bufs=1, space="PSUM") as psp,
            tc.tile_pool(name="zps", bufs=1, space="PSUM") as zpsp,
        ):
            # ---------------- stage A/B/C-early (transient f32 staging) ----
            with tc.tile_pool(name="stageA", bufs=1) as sa:
                xq_s = sa.tile([P, 4, NCHUNK], F32, name="xq_s")
                nc.sync.dma_start(xq_s[:], t["xqT"].rearrange("(t p) i -> p t i", p=P))
                xt_s = sa.tile([P, 4, S], F32, name="xt_s")
                nc.sync.dma_start(xt_s[:], t["xT"].rearrange("(t p) j -> p t j", p=P))

                # exact a/c (fp32 matmuls)
                at_ps = psp.tile([P, NCHUNK], F32, tag="atps", name="at_ps")
                for dt_ in range(4):
                    nc.tensor.matmul(
                        at_ps[:], mq_s[:, dt_, :], xq_s[:, dt_, :],
                        start=(dt_ == 0), stop=(dt_ == 3),
                    )
                nc.vector.tensor_scalar(at_s[:], at_ps[:], b1_s[:], None, AL.add)

                ct_ps = psp.tile([P, S], F32, tag="bigps", name="ct_ps")
                for jc in range(2):
                    for dt_ in range(4):
                        nc.tensor.matmul(
                            ct_ps[:, sl512[jc]], mk_s[:, dt_, :], xt_s[:, dt_, sl512[jc]],
                            start=(dt_ == 0), stop=(dt_ == 3),
                        )
                nc.scalar.copy(ct_s[:], ct_ps[:])

                # f32r conversions
                nc.vector.tensor_copy(xqr_s[:], xq_s[:])
                nc.vector.tensor_copy(xtr_s[:], xt_s[:])
                for wi, (nm, dst) in enumerate(
                    [("wq", wqr_s), ("wk", wkr_s), ("wv", wvr_s), ("wo", wor_s)]
                ):
                    ws = sa.tile([P, 4, D], F32, tag="wstage", bufs=2, name=f"ws_{nm}")
                    nc.sync.dma_start(ws[:], t[nm].rearrange("(t p) n -> p t n", p=P))
                    if wi % 2 == 0:
                        nc.vector.tensor_copy(dst[:], ws[:])
                    else:
                        nc.scalar.copy(dst[:], ws[:])

                # Q^T (f32r) + 1/16 scale folded into the PSUM->SBUF copy
                for dout in range(4):
                    qt_ps = psp.tile([P, NCHUNK], F32, tag="atps", name="qt_ps")
                    for dt_ in range(4):
                        nc.tensor.matmul(
                            qt_ps[:], wqr_s[:, dt_, 128 * dout : 128 * (dout + 1)],
                            xqr_s[:, dt_, :], start=(dt_ == 0), stop=(dt_ == 3),
                        )
                    nc.scalar.mul(qt_s[:, dout, :], qt_ps[:], 1.0 / 16.0)

            # ---------------- stage D + C + E + F ----------------
            with (
                tc.tile_pool(name="Tp", bufs=CONFIG["t_bufs"]) as Tp,
                tc.tile_pool(name="work", bufs=2) as work,
            ):
                # ---- stage D: z + mask ----
                mask_s = []
                for blk in range(2):
                    zp = zpsp.tile([P, S], F32, tag="z", name=f"zp{blk}")
                    for step in range(128):
                        if CONFIG["coltile"]:
                            k, g = step // 4, step % 4
                            i = 32 * g + k
                        else:
                            k, g, i = step, 0, step
                        ii = blk * 128 + i
                        T = Tp.tile([P, S], F32, tag="T", name=f"T{ii}")
                        eng = CONFIG["tpat"][ii % 16]
                        if eng == "v":
                            nc.vector.tensor_scalar(
                                T[:], ct_s[:], at_s[:, ii : ii + 1], 0.0, AL.add, AL.max
                            )
                        elif eng == "s":
                            nc.scalar.activation(
                                T[:], ct_s[:], AF.Relu, bias=at_s[:, ii : ii + 1]
                            )
                        else:
                            nc.gpsimd.tensor_scalar(
                                T[:], ct_s[:], at_s[:, ii : ii + 1], 0.0, AL.add, AL.max
                            )
                        if CONFIG["coltile"]:
                            for jc in range(2):
                                nc.tensor.matmul(
                                    zp[32 * g : 32 * g + 32, sl512[jc]],
                                    wsel_s[:, 32 - k : 64 - k],
                                    T[:, sl512[jc]],
                                    start=(k == 0), stop=(k == 31),
                                    tile_position=(0, 32 * g),
                                    skip_group_check=True,
                                )
                        else:
                            for jc in range(2):
                                nc.tensor.matmul(
                                    zp[:, sl512[jc]],
                                    wself_s[:, 128 - i : 256 - i],
                                    T[:, sl512[jc]],
                                    start=(i == 0), stop=(i == 127),
                                )
                    m01 = big.tile([P, S], F32, name=f"mask{blk}")
                    for jc in range(2):
                        nc.vector.tensor_scalar(
                            m01[:, sl512[jc]], zp[:, sl512[jc]], thr_s[:], None, AL.is_gt
                        )
                    mask_s.append(m01)

                # ---- stage C: K/V projections (f32r), fills PE gaps in D ----
                for dout in range(4):
                    kt_ps = psp.tile([P, S], F32, tag="bigps", name="kt_ps")
                    for jc in range(2):
                        for dt_ in range(4):
                            nc.tensor.matmul(
                                kt_ps[:, sl512[jc]],
                                wkr_s[:, dt_, 128 * dout : 128 * (dout + 1)],
                                xtr_s[:, dt_, sl512[jc]],
                                start=(dt_ == 0), stop=(dt_ == 3),
                            )
                    if dout % 2 == 0:
                        nc.vector.tensor_copy(kt_s[:, dout, :], kt_ps[:])
                    else:
                        nc.scalar.copy(kt_s[:, dout, :], kt_ps[:])

                for jt in range(8):
                    v_ps = psp.tile([P, D], F32, tag="vps", name="v_ps")
                    for dt_ in range(4):
                        nc.tensor.matmul(
                            v_ps[:], xtr_s[:, dt_, 128 * jt : 128 * (jt + 1)],
                            wvr_s[:, dt_, :], start=(dt_ == 0), stop=(dt_ == 3),
                        )
                    if jt % 2 == 0:
                        nc.vector.tensor_copy(v_s[:, jt, :], v_ps[:])
                    else:
                        nc.scalar.copy(v_s[:, jt, :], v_ps[:])

                # ---- stage E: attention ----
                for h in range(2):
                    attns = []
                    for ti in range(2):
                        sc_ps = psp.tile([P, S], F32, tag="bigps", name="sc_ps")
                        for jc in range(2):
                            for dt_ in range(2):
                                nc.tensor.matmul(
                                    sc_ps[:, sl512[jc]],
                                    qt_s[:, 2 * h + dt_, 128 * ti : 128 * (ti + 1)],
                                    kt_s[:, 2 * h + dt_, sl512[jc]],
                                    start=(dt_ == 0), stop=(dt_ == 1),
                                )
                        negm = work.tile([P, 1], F32, tag="negm", name="negm")
                        nc.vector.reduce_max(
                            negm[:], sc_ps[:], axis=mybir.AxisListType.X, negate=True
                        )
                        e = work.tile([P, S], F32, tag="e", name="e")
                        nc.scalar.activation(e[:], sc_ps[:], AF.Exp, bias=negm[:])
                        em = work.tile([P, S], F32, tag="em", name="em")
                        ssum = work.tile([P, 1], F32, tag="ssum", name="ssum")
                        nc.vector.tensor_mul(em[:], e[:], mask_s[ti][:])
                        nc.vector.reduce_sum(ssum[:], em[:], axis=mybir.AxisListType.X)
                        # fully-masked rows: reference = uniform 1/1024.
                        # ind = [s==0]; attn = (em + ind) / (s + 1024*ind)
                        ind = work.tile([P, 1], F32, tag="ind", name="ind")
                        nc.vector.tensor_scalar(ind[:], ssum[:], 0.0, None, AL.is_equal)
                        s2 = work.tile([P, 1], F32, tag="s2", name="s2")
                        nc.vector.tensor_scalar(s2[:], ind[:], 1024.0, ssum[:], AL.mult, AL.add)
                        rinv = work.tile([P, 1], F32, tag="rinv", name="rinv")
                        nc.vector.reciprocal(rinv[:], s2[:])
                        attn = work.tile([P, S], F32, tag="attn", name="attn")
                        nc.vector.tensor_scalar(attn[:], em[:], ind[:], rinv[:], AL.add, AL.mult)
                        attns.append(attn)
                    att_sb = []
                    for jt in range(8):
                        tp_ps = psp.tile([P, NCHUNK], F32, tag="tp", bufs=2, name="tp_ps")
                        for ti in range(2):
                            nc.tensor.transpose(
                                tp_ps[:, 128 * ti : 128 * (ti + 1)],
                                attns[ti][:, 128 * jt : 128 * (jt + 1)],
                                ident[:],
                            )
                        a_sb = work.tile([P, NCHUNK], F32R, tag="attnT", bufs=8, name="a_sb")
                        if jt % 2 == 0:
                            nc.vector.tensor_copy(a_sb[:], tp_ps[:])
                        else:
                            nc.scalar.copy(a_sb[:], tp_ps[:])
                        att_sb.append(a_sb)
                    for dt_ in range(2):
                        ot_ps = psp.tile([P, NCHUNK], F32, tag="tp", bufs=2, name="ot_ps")
                        for jt in range(8):
                            nc.tensor.matmul(
                                ot_ps[:],
                                v_s[:, jt, 256 * h + 128 * dt_ : 256 * h + 128 * (dt_ + 1)],
                                att_sb[jt][:],
                                start=(jt == 0), stop=(jt == 7),
                            )
                        if dt_ == 0:
                            nc.vector.tensor_copy(otr_s[:, 2 * h + dt_, :], ot_ps[:])
                        else:
                            nc.scalar.copy(otr_s[:, 2 * h + dt_, :], ot_ps[:])

                # ---- stage F: output projection ----
                for ti in range(2):
                    o_ps = psp.tile([P, D], F32, tag="vps", name="o_ps")
                    nc.tensor.matmul(o_ps[:], oner_s[:], bor_s[:], start=True, stop=False)
                    for dt_ in range(4):
                        nc.tensor.matmul(
                            o_ps[:], otr_s[:, dt_, 128 * ti : 128 * (ti + 1)],
                            wor_s[:, dt_, :], start=False, stop=(dt_ == 3),
                        )
                    o_sb = work.tile([P, D], F32, tag="osb", name="o_sb")
                    nc.vector.tensor_copy(o_sb[:], o_ps[:])
                    nc.sync.dma_start(t["out"][128 * ti : 128 * (ti + 1), :], o_sb[:])


def _build():
    if "nc" in _STATE:
        return _STATE["nc"]
    nc = bacc.Bacc(
        "TRN2", target_bir_lowering=False, debug=False, enable_asserts=True,
        num_devices=8,
    )
    t = {}
    t["xT"] = nc.dram_tensor("xT", [D, S], F32, kind="ExternalInput").ap()
    t["xqT"] = nc.dram_tensor("xqT", [D, NCHUNK], F32, kind="ExternalInput").ap()
    t["wq"] = nc.dram_tensor("wq", [D, D], F32, kind="ExternalInput").ap()
    t["wk"] = nc.dram_tensor("wk", [D, D], F32, kind="ExternalInput").ap()
    t["wv"] = nc.dram_tensor("wv", [D, D], F32, kind="ExternalInput").ap()
    t["wo"] = nc.dram_tensor("wo", [D, D], F32, kind="ExternalInput").ap()
    t["mq"] = nc.dram_tensor("mq", [D, HID], F32, kind="ExternalInput").ap()
    t["mk"] = nc.dram_tensor("mk", [D, HID], F32, kind="ExternalInput").ap()
    t["b1c"] = nc.dram_tensor("b1c", [P, 1], F32, kind="ExternalInput").ap()
    t["thr"] = nc.dram_tensor("thr", [P, 1], F32, kind="ExternalInput").ap()
    t["wsel32"] = nc.dram_tensor("wsel32", [P, 64], F32, kind="ExternalInput").ap()
    if not CONFIG["coltile"]:
        t["wself"] = nc.dram_tensor("wself", [P, 256], F32, kind="ExternalInput").ap()
    t["bo2"] = nc.dram_tensor("bo2", [1, D], F32, kind="ExternalInput").ap()
    t["one1"] = nc.dram_tensor("one1", [1, P], F32, kind="ExternalInput").ap()
    t["out"] = nc.dram_tensor("out", [NCHUNK, D], F32, kind="ExternalOutput").ap()

    with tile.TileContext(nc) as tc:
        _emit(tc, nc, t)
    nc.compile()
    _STATE["nc"] = nc
    return nc


def _prep_in_maps(inputs):
    x = np.ascontiguousarray(np.asarray(inputs["x"], np.float32))
    Wq = np.asarray(inputs["Wq"], np.float32)
    Wk = np.asarray(inputs["Wk"], np.float32)
    Wv = np.asarray(inputs["Wv"], np.float32)
    Wo = np.asarray(inputs["Wo"], np.float32)
    bo = np.asarray(inputs["bo"], np.float32)
    W1 = np.asarray(inputs["W1"], np.float64)
    b1 = np.asarray(inputs["b1"], np.float32)
    W2 = np.asarray(inputs["W2"], np.float32)
    b2 = np.asarray(inputs["b2"], np.float32)

    wq_m = 0.5 * (Wq[:, :DH].astype(np.float64) + Wq[:, DH:].astype(np.float64))
    wk_m = 0.5 * (Wk[:, :DH].astype(np.float64) + Wk[:, DH:].astype(np.float64))
    Mq = np.ascontiguousarray((wq_m @ W1[:DH]).astype(np.float32))
    Mk = np.ascontiguousarray((wk_m @ W1[DH:]).astype(np.float32))

    wsel32 = np.zeros((P, 64), np.float32)
    wsel32[:, 32] = W2[:, 0]
    b1c = np.ascontiguousarray(b1.reshape(P, 1))
    thr = np.full((P, 1), -float(b2[0]), np.float32)
    bo2 = np.ascontiguousarray(bo.reshape(1, D))
    one1 = np.ones((1, P), np.float32)

    shared = dict(
        wq=Wq, wk=Wk, wv=Wv, wo=Wo, mq=Mq, mk=Mk, b1c=b1c, thr=thr,
        wsel32=wsel32, bo2=bo2, one1=one1,
    )
    if not CONFIG["coltile"]:
        wself = np.zeros((P, 256), np.float32)
        wself[:, 128] = W2[:, 0]
        shared["wself"] = wself
    in_maps = []
    xT = [np.ascontiguousarray(x[b].T) for b in range(B)]
    for c in range(8):
        b, i0 = c // 4, (c % 4) * NCHUNK
        m = dict(shared)
        m["xT"] = xT[b]
        m["xqT"] = np.ascontiguousarray(x[b, i0 : i0 + NCHUNK].T)
        in_maps.append(m)
    return in_maps


def kernel(**inputs):
    nc = _build()
    in_maps = _prep_in_maps(inputs)
    res = run_bass_kernel_spmd(
        nc, in_maps, core_ids=list(range(8)),
        trace=CONFIG["trace"], tmpdir=CONFIG["tmpdir"],
    )
    _STATE["last_result"] = res
    out = np.empty((B, S, D), np.float32)
    for c in range(8):
        b, i0 = c // 4, (c % 4) * NCHUNK
        out[b, i0 : i0 + NCHUNK] = res.results[c]["out"]
    return out
